# revision 44
# baseline (speedup 1.0000x reference)
"""Trainium2 Bass kernel for AdaptiveGraphConv (per-(b,t) graph attention + BatchNorm2d).

Reference math (B=8, C=256, T=64, V=468, INTER=128, OUT=256):
    theta = einsum('bctv,ic->btvi', x, W_theta) + b_theta
    phi   = einsum('bctv,ic->btvi', x, W_phi)   + b_phi
    g     = einsum('bctv,oc->btvo', x, W_g)     + b_g
    A     = softmax(theta @ phi^T / sqrt(INTER), axis=-1)   # per (b,t), V x V
    out   = (A @ g) transposed to (B, OUT, T, V)
    out   = batchnorm2d(out, training stats over (B,T,V) per channel)

Sharding: data-parallel over B (1 batch per NeuronCore, 8 cores);
BN batch statistics are all-reduced across cores.

Device program (SPMD, per core, matmuls bf16 with fp32 PSUM accumulation):
  phase 1 (per t slice):
    thetaT[i,v], phiT[i,v] = W^T-chunks x x-chunks           (PE)
    S^T[w,v] = phiT-cols^T @ thetaT ; E^T = exp(S^T/sqrt(d)) (PE + ACT)
    g[w,o] = x-cols^T @ WgT ; cast to bf16                   (PE + ACT)
    esum = sum of E^T chunks (partial-row chunk 3 at K=84,
           no zero padding needed)                           (DVE)
    U^T[o,v] = g-cols^T @ E^T  (K=true chunk size)           (PE)
    Z = ones-matmul partition reduce, emitted after U        (PE)
    zinv = 1/Z; o = U^T * zinv -> SBUF-resident bf16 and
    bn_stats, deferred one t-slice to avoid DVE FIFO
    head-blocking on the z-matmul                            (DVE)
  BN stats: partial bn_aggr over t<48 hidden in phase 1; tail
  aggregates t in [48,64), sums payloads, all-reduce, affine coeffs.
  phase 2: SBUF-resident o -> out*s+t per channel -> bf16 output DMA.
  Host casts the bf16 output back to float32.
"""

import math

import numpy as np
import ml_dtypes

import concourse.bacc as bacc
import concourse.tile as tile
from concourse import mybir
from concourse.bass_utils import run_bass_kernel_spmd

B, C, T, V = 8, 256, 64, 468
INTER, OUT = 128, 256
BN_EPS = 1e-5
NCORES = 8
P = 128

SCALE = 1.0 / math.sqrt(INTER)
# w-axis chunks of V for 128-partition tiles
WCH = [(0, 128), (128, 128), (256, 128), (384, V - 384)]
T_BLK = 8  # t-slices per input DMA / phase-2 tile
N_TB = T // T_BLK
# BN stats for t<56 aggregate in hidden blocks during phase 1; only
# t in [56,64) aggregates in the tail

F32 = mybir.dt.float32
BF16 = mybir.dt.bfloat16

TRACE = False
LAST_EXEC_NS = None

_CACHE = {}


def _build(with_bias: bool):
    nc = bacc.Bacc("TRN2", target_bir_lowering=False, debug=False, num_devices=NCORES)

    x_ext = nc.dram_tensor("x", [C, T, V], BF16, kind="ExternalInput").ap()
    wt_ext = nc.dram_tensor("wt", [2, P, INTER], BF16, kind="ExternalInput").ap()
    wp_ext = nc.dram_tensor("wp", [2, P, INTER], BF16, kind="ExternalInput").ap()
    wg_ext = nc.dram_tensor("wg", [2, P, OUT], BF16, kind="ExternalInput").ap()
    gb_ext = nc.dram_tensor("gb", [P, 4], F32, kind="ExternalInput").ap()
    if with_bias:
        bt_ext = nc.dram_tensor("bt", [INTER, 1], F32, kind="ExternalInput").ap()
        bp_ext = nc.dram_tensor("bp", [INTER, 1], F32, kind="ExternalInput").ap()
        bg_ext = nc.dram_tensor("bg", [1, OUT], F32, kind="ExternalInput").ap()
    out_ext = nc.dram_tensor("out", [OUT, T, V], BF16, kind="ExternalOutput").ap()

    cnt_glob = float(NCORES * T * V)

    with tile.TileContext(nc) as tc:
        with (
            tc.tile_pool(name="consts", bufs=1) as consts,
            tc.tile_pool(name="xin", bufs=2) as xin,
            tc.tile_pool(name="thp", bufs=4) as thp,
            tc.tile_pool(name="ep", bufs=8) as ep,
            tc.tile_pool(name="gp", bufs=8) as gp,
            tc.tile_pool(name="esp", bufs=6) as esp,
            tc.tile_pool(name="zp", bufs=2) as zp,
            tc.tile_pool(name="small", bufs=1) as small,
            tc.tile_pool(name="pp_th", bufs=2, space="PSUM") as pp_th,
            tc.tile_pool(name="pp_sg", bufs=3, space="PSUM") as pp_sg,
            tc.tile_pool(name="pp_z", bufs=1, space="PSUM") as pp_z,
            tc.tile_pool(name="pp_u", bufs=2, space="PSUM") as pp_u,
            tc.tile_pool(name="dram", bufs=1, space="DRAM") as dram,
        ):
            # ---- constants ----
            wt_sb = [consts.tile([P, INTER], BF16, tag=f"wt{k}", name=f"wt_sb{k}") for k in range(2)]
            wp_sb = [consts.tile([P, INTER], BF16, tag=f"wp{k}", name=f"wp_sb{k}") for k in range(2)]
            wg_sb = [consts.tile([P, OUT], BF16, tag=f"wg{k}", name=f"wg_sb{k}") for k in range(2)]
            for k in range(2):
                nc.sync.dma_start(out=wt_sb[k][:], in_=wt_ext[k])
                nc.sync.dma_start(out=wp_sb[k][:], in_=wp_ext[k])
                nc.sync.dma_start(out=wg_sb[k][:], in_=wg_ext[k])
            ones = consts.tile([P, P], BF16, tag="ones")
            nc.vector.memset(ones[:], 1.0)
            gb_sb = consts.tile([P, 4], F32, tag="gb")
            nc.sync.dma_start(out=gb_sb[:], in_=gb_ext[:])
            eps_sb = consts.tile([P, 1], F32, tag="eps")
            nc.vector.memset(eps_sb[:], BN_EPS)
            warm = consts.tile([P, 1], F32, tag="warm")
            nc.scalar.activation(warm[:], eps_sb[:], mybir.ActivationFunctionType.Exp)

            # PE warm-up spinner: dummy matmuls on the ones tile run during
            # the NEFF preamble + first input DMA, so the HAM clock gate is
            # already open (and stays open) when the first real matmul issues
            warm_ps = pp_z.tile([P, P], F32, tag="z", name="warm_ps")
            for _ in range(64):
                nc.tensor.matmul(
                    warm_ps[:], lhsT=ones[:], rhs=ones[:], start=True, stop=True
                )
            if with_bias:
                bt_sb = consts.tile([INTER, 1], F32, tag="bt")
                bp_sb = consts.tile([INTER, 1], F32, tag="bp")
                bg_sb = consts.tile([P, OUT], F32, tag="bg")
                nc.sync.dma_start(out=bt_sb[:], in_=bt_ext[:])
                nc.sync.dma_start(out=bp_sb[:], in_=bp_ext[:])
                nc.sync.dma_start(out=bg_sb[:], in_=bg_ext.to_broadcast([P, OUT]))

            # per-channel running stats (bn_stats 6-tuples per t-slice and o-chunk)
            stats_acc_t = consts.tile([P, T, 2, 6], F32, tag="stats", name="stats_acc_t")

            # all phase-1 outputs stay SBUF-resident (no DRAM scratch round-trip)
            obig = {
                (oc, tb): consts.tile([P, T_BLK, V], BF16, tag=f"obig{oc}_{tb}",
                                      name=f"obig{oc}_{tb}")
                for oc in range(2) for tb in range(N_TB)
            }

            payA = small.tile([P, 4], F32, tag="payA")
            payB = small.tile([P, 4], F32, tag="payB")
            payAB = small.tile([P, 4], F32, tag="payAB")
            payABC = small.tile([P, 4], F32, tag="payABC")
            pay_blk = {
                tag: small.tile([P, 4], F32, tag=f"payblk{tag}", name=f"payblk{tag}")
                for tag in ("A", "B", "C", "D")
            }

            # tiny warm-up AllReduce, issued early in phase 1: pre-stages the
            # collective path and absorbs cross-core launch skew while the
            # compute engines are busy, so the real stats AllReduce at the
            # end pays minimal latency
            warm_pay = dram.tile([P, 1], F32, name="warm_pay")
            warm_red = dram.tile([P, 1], F32, name="warm_red")

            def warm_collective(src):
                # src controls WHEN this fires: the collective queue idles
                # until src's producer has run, so passing phase-1 data
                # pins the sync point into the middle of phase 1
                nc.sync.dma_start(out=warm_pay[:], in_=src)
                nc.gpsimd.collective_compute(
                    "AllReduce",
                    mybir.AluOpType.add,
                    replica_groups=[list(range(NCORES))],
                    ins=[warm_pay.opt()],
                    outs=[warm_red.opt()],
                )

            stats_mv = {}

            def stats_aggr(key, t0, t1, oc):
                mv = small.tile([P, 2], F32, tag=f"mv{t0}_{oc}", name=f"mv{t0}_{oc}")
                nc.vector.bn_aggr(out=mv[:], in_=stats_acc_t[:, t0:t1, oc, :])
                stats_mv[key] = mv

            def stats_scale(pay, key, t0, t1, oc):
                # pay[:, oc] = sum, pay[:, 2+oc] = sumsq over t in [t0, t1)
                cnt = float((t1 - t0) * V)
                mv = stats_mv[key]
                t1_ = small.tile([P, 1], F32, tag=f"tmp1{t0}_{oc}", name=f"t1_{t0}_{oc}")
                t2_ = small.tile([P, 1], F32, tag=f"tmp2{t0}_{oc}", name=f"t2_{t0}_{oc}")
                nc.vector.tensor_scalar_mul(pay[:, oc : oc + 1], mv[:, 0:1], cnt)
                nc.vector.tensor_tensor(
                    t1_[:], mv[:, 0:1], mv[:, 0:1], mybir.AluOpType.mult
                )
                nc.vector.tensor_tensor(
                    t2_[:], mv[:, 1:2], t1_[:], mybir.AluOpType.add
                )
                nc.vector.tensor_scalar_mul(pay[:, 2 + oc : 3 + oc], t2_[:], cnt)

            pending = [None]

            def flush_pending():
                if pending[0] is None:
                    return
                z_ps, u_pss, ptb, ptt, pt = pending[0]
                pending[0] = None
                zinv = zp.tile([P, V], F32, tag="zinv")
                nc.vector.reciprocal_approx_fast(out=zinv[:], in_=z_ps[:])
                for oc in range(2):
                    o_ap = obig[(oc, ptb)][:, ptt, :]
                    nc.vector.tensor_tensor(
                        o_ap, u_pss[oc][:], zinv[:], mybir.AluOpType.mult
                    )
                    nc.vector.bn_stats(
                        out=stats_acc_t[:, pt, oc, :], in_=o_ap
                    )

            # ---- phase 1 ----
            for tb in range(N_TB):
                xg = [xin.tile([P, T_BLK, V], BF16, tag=f"xg{k}", name=f"xg{k}") for k in range(2)]
                if tb == 0:
                    # fine-grained first block so the first matmul starts as
                    # soon as the first t-slice lands instead of after ~1MB
                    for tt in range(T_BLK):
                        for k in range(2):
                            nc.sync.dma_start(
                                out=xg[k][:, tt, :],
                                in_=x_ext[k * P : (k + 1) * P, tt, :],
                            )
                else:
                    for k in range(2):
                        nc.sync.dma_start(
                            out=xg[k][:],
                            in_=x_ext[k * P : (k + 1) * P, tb * T_BLK : (tb + 1) * T_BLK, :],
                        )
                for tt in range(T_BLK):
                    t = tb * T_BLK + tt
                    xt = [xg[k][:, tt, :] for k in range(2)]

                    # thetaT / phiT : [INTER, V]
                    th_ps = pp_th.tile([P, V], F32, tag="th")
                    ph_ps = pp_th.tile([P, V], F32, tag="th")
                    for k in range(2):
                        nc.tensor.matmul(
                            th_ps[:], lhsT=wt_sb[k][:], rhs=xt[k],
                            start=(k == 0), stop=(k == 1),
                        )
                    for k in range(2):
                        nc.tensor.matmul(
                            ph_ps[:], lhsT=wp_sb[k][:], rhs=xt[k],
                            start=(k == 0), stop=(k == 1),
                        )
                    tp_sb = thp.tile([P, 2, V], BF16, tag="th_sb")
                    if with_bias:
                        nc.scalar.activation(
                            tp_sb[:, 0, :], th_ps[:],
                            mybir.ActivationFunctionType.Identity,
                            bias=bt_sb[:, 0:1],
                        )
                        nc.scalar.activation(
                            tp_sb[:, 1, :], ph_ps[:],
                            mybir.ActivationFunctionType.Identity,
                            bias=bp_sb[:, 0:1],
                        )
                    else:
                        # split across engines: th on ACT, ph on DVE
                        nc.scalar.copy(tp_sb[:, 0, :], th_ps[:])
                        nc.vector.tensor_copy(tp_sb[:, 1, :], ph_ps[:])
                    th_sb = tp_sb[:, 0, :]
                    ph_sb = tp_sb[:, 1, :]

                    # scores^T chunks + exp; chunk 3 stays at its true 84 rows
                    # (U matmuls use K=wsz, so no zero padding is ever read)
                    e_sb = []
                    for wc, (w0, wsz) in enumerate(WCH):
                        s_ps = pp_sg.tile([P, V], F32, tag="sg")
                        nc.tensor.matmul(
                            s_ps[:wsz], lhsT=ph_sb[:, w0 : w0 + wsz], rhs=th_sb,
                            start=True, stop=True,
                        )
                        e_t = ep.tile([P, V], BF16, tag="e")
                        nc.scalar.activation(
                            e_t[:wsz], s_ps[:wsz],
                            mybir.ActivationFunctionType.Exp, scale=SCALE,
                        )
                        e_sb.append(e_t)

                    # g chunks: [w, OUT], two chunks share one PSUM bank
                    g_sb = []
                    for pair in range(2):
                        gp_ps = pp_sg.tile([P, 2, OUT], F32, tag="sg")
                        for j in range(2):
                            w0, wsz = WCH[pair * 2 + j]
                            for k in range(2):
                                nc.tensor.matmul(
                                    gp_ps[:wsz, j, :],
                                    lhsT=xt[k][:, w0 : w0 + wsz], rhs=wg_sb[k][:],
                                    start=(k == 0), stop=(k == 1),
                                )
                        g_t = gp.tile([P, 2, OUT], BF16, tag="g")
                        if with_bias:
                            for j in range(2):
                                wsz = WCH[pair * 2 + j][1]
                                nc.vector.tensor_tensor(
                                    g_t[:wsz, j, :], gp_ps[:wsz, j, :], bg_sb[:wsz],
                                    mybir.AluOpType.add,
                                )
                        else:
                            # one full-pair ACT copy (rows past 84 in chunk 3 are
                            # junk but never read: U matmuls contract K=wsz)
                            nc.scalar.copy(g_t[:], gp_ps[:])
                        g_sb.append(g_t[:, 0, :])
                        g_sb.append(g_t[:, 1, :])

                    # softmax denominator: pre-fold E chunks pairwise on DVE,
                    # then a ones-matmul reduces over partitions + broadcasts
                    wsz3 = WCH[3][1]
                    e01 = esp.tile([P, V], BF16, tag="esum")
                    nc.vector.tensor_tensor(
                        e01[:], e_sb[0][:], e_sb[1][:], mybir.AluOpType.add
                    )
                    e23 = esp.tile([P, V], BF16, tag="esum")
                    # partition offsets must be 32-aligned; rows [64:84] are
                    # overwritten by the add below
                    nc.vector.tensor_copy(e23[64:], e_sb[2][64:])
                    nc.vector.tensor_tensor(
                        e23[:wsz3], e_sb[2][:wsz3], e_sb[3][:wsz3],
                        mybir.AluOpType.add,
                    )
                    esum = esp.tile([P, V], BF16, tag="esum")
                    nc.vector.tensor_tensor(
                        esum[:], e01[:], e23[:], mybir.AluOpType.add
                    )

                    # normalization + stats of the PREVIOUS t-slice: deferring
                    # these one iteration keeps the DVE FIFO from head-blocking
                    # on the z-matmul (its z_ps input is long done by now)
                    flush_pending()

                    # U^T = A_unnorm @ g : [OUT(2x128), V]
                    u_pss = []
                    for oc in range(2):
                        u_ps = pp_u.tile([P, V], F32, tag="u")
                        for wc, (w0, wsz) in enumerate(WCH):
                            nc.tensor.matmul(
                                u_ps[:],
                                lhsT=g_sb[wc][:wsz, oc * P : (oc + 1) * P],
                                rhs=e_sb[wc][:wsz, :],
                                start=(wc == 0), stop=(wc == 3),
                            )
                        u_pss.append(u_ps)

                    # z-matmul after the U matmuls: PE is in-order, so a wait
                    # on esum here no longer blocks the U matmuls
                    z_ps = pp_z.tile([P, V], F32, tag="z")
                    nc.tensor.matmul(
                        z_ps[:], lhsT=ones[:], rhs=esum[:], start=True, stop=True
                    )
                    pending[0] = (z_ps, u_pss, tb, tt, t)

                    if t == T_BLK - 1:
                        warm_collective(eps_sb[:])
                    # aggregate stats in blocks while phase 1 continues,
                    # staggered across t iterations to avoid DVE bursts;
                    # only t in [56,64) is left for the tail
                    for b0, b1, tag, base, step in (
                        (0, 16, "A", 17, 2),
                        (16, 32, "B", 33, 2),
                        (32, 48, "C", 49, 2),
                        (48, 56, "D", 57, 1),
                    ):
                        if t == base:
                            stats_aggr(f"{tag}0", b0, b1, 0)
                        elif t == base + step:
                            stats_aggr(f"{tag}1", b0, b1, 1)
                        elif t == base + 2 * step:
                            stats_scale(pay_blk[tag], f"{tag}0", b0, b1, 0)
                        elif t == base + 3 * step:
                            stats_scale(pay_blk[tag], f"{tag}1", b0, b1, 1)
                    if t == 42:
                        nc.vector.tensor_tensor(
                            payAB[:], pay_blk["A"][:], pay_blk["B"][:],
                            mybir.AluOpType.add,
                        )
                    elif t == 56:
                        nc.vector.tensor_tensor(
                            payABC[:], payAB[:], pay_blk["C"][:],
                            mybir.AluOpType.add,
                        )
                    elif t == 62:
                        nc.vector.tensor_tensor(
                            payA[:], payABC[:], pay_blk["D"][:],
                            mybir.AluOpType.add,
                        )

            flush_pending()

            # ---- BN stats tail: aggregate t in [56,64), sum, all-reduce ----
            for oc in range(2):
                stats_aggr(f"E{oc}", 56, T, oc)
                stats_scale(payB, f"E{oc}", 56, T, oc)
            pay = small.tile([P, 4], F32, tag="pay")
            nc.vector.tensor_tensor(pay[:], payA[:], payB[:], mybir.AluOpType.add)

            pay_dram = dram.tile([P, 4], F32)
            red_dram = dram.tile([P, 4], F32)
            nc.sync.dma_start(out=pay_dram[:], in_=pay[:])
            nc.gpsimd.collective_compute(
                "AllReduce",
                mybir.AluOpType.add,
                replica_groups=[list(range(NCORES))],
                ins=[pay_dram.opt()],
                outs=[red_dram.opt()],
            )
            red = small.tile([P, 4], F32, tag="red")
            nc.sync.dma_start(out=red[:], in_=red_dram[:])

            # mean = sum/N ; var = sumsq/N - mean^2 ; s = gamma/sqrt(var+eps)
            # t = beta - mean*s
            mean_g = small.tile([P, 2], F32, tag="mean_g")
            ex2 = small.tile([P, 2], F32, tag="ex2")
            var_g = small.tile([P, 2], F32, tag="var_g")
            rstd = small.tile([P, 2], F32, tag="rstd")
            s_vec = small.tile([P, 2], F32, tag="s_vec")
            t_vec = small.tile([P, 2], F32, tag="t_vec")
            nc.vector.tensor_scalar_mul(mean_g[:], red[:, 0:2], 1.0 / cnt_glob)
            nc.vector.tensor_scalar_mul(ex2[:], red[:, 2:4], 1.0 / cnt_glob)
            nc.vector.tensor_tensor(
                var_g[:], mean_g[:], mean_g[:], mybir.AluOpType.mult
            )
            nc.vector.tensor_tensor(
                var_g[:], ex2[:], var_g[:], mybir.AluOpType.subtract
            )
            nc.scalar.activation(
                rstd[:], var_g[:], mybir.ActivationFunctionType.Sqrt,
                bias=eps_sb[:, 0:1],
            )
            nc.vector.reciprocal(out=rstd[:], in_=rstd[:])
            nc.vector.tensor_tensor(s_vec[:], rstd[:], gb_sb[:, 0:2], mybir.AluOpType.mult)
            nc.vector.tensor_tensor(t_vec[:], mean_g[:], s_vec[:], mybir.AluOpType.mult)
            nc.vector.tensor_tensor(
                t_vec[:], gb_sb[:, 2:4], t_vec[:], mybir.AluOpType.subtract
            )

            # ---- phase 2: SBUF-resident o -> affine in place -> bf16 out ----
            # the affine runs in place on obig, so the output DMA streams
            # straight from it with no staging tiles in between
            half = T_BLK // 2
            for tb in range(N_TB):
                for oc in range(2):
                    t0 = tb * T_BLK
                    tin = obig[(oc, tb)]
                    for h in range(2):
                        nc.vector.tensor_scalar(
                            tin[:, h * half : (h + 1) * half, :],
                            tin[:, h * half : (h + 1) * half, :],
                            s_vec[:, oc : oc + 1], t_vec[:, oc : oc + 1],
                            mybir.AluOpType.mult, mybir.AluOpType.add,
                        )
                        nc.sync.dma_start(
                            out=out_ext[oc * P : (oc + 1) * P,
                                        t0 + h * half : t0 + (h + 1) * half, :],
                            in_=tin[:, h * half : (h + 1) * half, :],
                        )

    nc.compile()
    return nc


def _get_nc(with_bias: bool):
    key = with_bias
    if key not in _CACHE:
        _CACHE[key] = _build(with_bias)
    return _CACHE[key]


def _ensure_ntff_hook():
    import sys, types
    import antenv

    if "antenv.axon_hooks" not in sys.modules:
        mod = types.ModuleType("antenv.axon_hooks")
        _h = [None]
        mod.set_axon_ntff_profile_hook = lambda h: _h.__setitem__(0, h)
        mod.get_axon_ntff_profile_hook = lambda: _h[0]
        sys.modules["antenv.axon_hooks"] = mod
        antenv.axon_hooks = mod
    mod = sys.modules["antenv.axon_hooks"]
    if mod.get_axon_ntff_profile_hook() is None:
        try:
            from trn_agent_boot.trn_boot import _ntff_profile_via_ctypes

            mod.set_axon_ntff_profile_hook(
                _ntff_profile_via_ctypes("/opt/axon/libaxon_pjrt.so")
            )
        except Exception:
            pass


def kernel(x, W_theta, b_theta, W_phi, b_phi, W_g, b_g, bn_gamma, bn_beta):
    global LAST_EXEC_NS
    x = np.asarray(x, dtype=np.float32)
    with_bias = bool(
        np.any(np.asarray(b_theta)) or np.any(np.asarray(b_phi)) or np.any(np.asarray(b_g))
    )

    x_bf = x.astype(ml_dtypes.bfloat16)  # (B, C, T, V)
    wt = np.ascontiguousarray(
        np.asarray(W_theta, dtype=np.float32).T.astype(ml_dtypes.bfloat16).reshape(2, P, INTER)
    )
    wp = np.ascontiguousarray(
        np.asarray(W_phi, dtype=np.float32).T.astype(ml_dtypes.bfloat16).reshape(2, P, INTER)
    )
    wg = np.ascontiguousarray(
        np.asarray(W_g, dtype=np.float32).T.astype(ml_dtypes.bfloat16).reshape(2, P, OUT)
    )
    gamma = np.asarray(bn_gamma, dtype=np.float32).reshape(2, P).T  # [128, 2]
    beta = np.asarray(bn_beta, dtype=np.float32).reshape(2, P).T
    gb = np.ascontiguousarray(np.concatenate([gamma, beta], axis=1))  # [128, 4]

    nc = _get_nc(with_bias)

    in_maps = []
    for b in range(NCORES):
        m = {
            "x": np.ascontiguousarray(x_bf[b]),
            "wt": wt,
            "wp": wp,
            "wg": wg,
            "gb": gb,
        }
        if with_bias:
            m["bt"] = np.asarray(b_theta, dtype=np.float32).reshape(INTER, 1)
            m["bp"] = np.asarray(b_phi, dtype=np.float32).reshape(INTER, 1)
            m["bg"] = np.asarray(b_g, dtype=np.float32).reshape(1, OUT)
        in_maps.append(m)

    if TRACE:
        _ensure_ntff_hook()
    r = run_bass_kernel_spmd(nc, in_maps, list(range(NCORES)), trace=TRACE)
    LAST_EXEC_NS = r.exec_time_ns

    out = np.stack([r.results[b]["out"] for b in range(NCORES)], axis=0)
    return out.astype(np.float32)


# revision 47
# speedup vs baseline: 1.0226x; 1.0226x over previous
"""Trainium2 Bass kernel for AdaptiveGraphConv (per-(b,t) graph attention + BatchNorm2d).

Reference math (B=8, C=256, T=64, V=468, INTER=128, OUT=256):
    theta = einsum('bctv,ic->btvi', x, W_theta) + b_theta
    phi   = einsum('bctv,ic->btvi', x, W_phi)   + b_phi
    g     = einsum('bctv,oc->btvo', x, W_g)     + b_g
    A     = softmax(theta @ phi^T / sqrt(INTER), axis=-1)   # per (b,t), V x V
    out   = (A @ g) transposed to (B, OUT, T, V)
    out   = batchnorm2d(out, training stats over (B,T,V) per channel)

Sharding: data-parallel over B (1 batch per NeuronCore, 8 cores);
BN batch statistics are all-reduced across cores.

Device program (SPMD, per core, matmuls bf16 with fp32 PSUM accumulation):
  phase 1 (per t slice):
    thetaT[i,v], phiT[i,v] = W^T-chunks x x-chunks           (PE)
    S^T[w,v] = phiT-cols^T @ thetaT ; E^T = exp(S^T/sqrt(d)) (PE + ACT)
    g[w,o] = x-cols^T @ WgT ; cast to bf16                   (PE + ACT)
    esum = sum of E^T chunks (partial-row chunk 3 at K=84,
           no zero padding needed)                           (DVE)
    U^T[o,v] = g-cols^T @ E^T  (K=true chunk size)           (PE)
    Z = ones-matmul partition reduce, emitted after U        (PE)
    zinv = 1/Z; o = U^T * zinv -> SBUF-resident bf16 and
    bn_stats, deferred one t-slice to avoid DVE FIFO
    head-blocking on the z-matmul                            (DVE)
  BN stats: partial bn_aggr over t<48 hidden in phase 1; tail
  aggregates t in [48,64), sums payloads, all-reduce, affine coeffs.
  phase 2: SBUF-resident o -> out*s+t per channel -> bf16 output DMA.
  Host casts the bf16 output back to float32.
"""

import math

import numpy as np
import ml_dtypes

import concourse.bacc as bacc
import concourse.tile as tile
from concourse import mybir
from concourse.bass_utils import run_bass_kernel_spmd

B, C, T, V = 8, 256, 64, 468
INTER, OUT = 128, 256
BN_EPS = 1e-5
NCORES = 8
P = 128

SCALE = 1.0 / math.sqrt(INTER)
# w-axis chunks of V for 128-partition tiles
WCH = [(0, 128), (128, 128), (256, 128), (384, V - 384)]
T_BLK = 8  # t-slices per input DMA / phase-2 tile
N_TB = T // T_BLK
# BN stats for t<56 aggregate in hidden blocks during phase 1; only
# t in [56,64) aggregates in the tail

F32 = mybir.dt.float32
BF16 = mybir.dt.bfloat16

TRACE = False
LAST_EXEC_NS = None

_CACHE = {}


def _build(with_bias: bool):
    nc = bacc.Bacc("TRN2", target_bir_lowering=False, debug=False, num_devices=NCORES)

    x_ext = nc.dram_tensor("x", [C, T, V], BF16, kind="ExternalInput").ap()
    wt_ext = nc.dram_tensor("wt", [2, P, INTER], BF16, kind="ExternalInput").ap()
    wp_ext = nc.dram_tensor("wp", [2, P, INTER], BF16, kind="ExternalInput").ap()
    wg_ext = nc.dram_tensor("wg", [2, P, OUT], BF16, kind="ExternalInput").ap()
    gb_ext = nc.dram_tensor("gb", [P, 4], F32, kind="ExternalInput").ap()
    if with_bias:
        bt_ext = nc.dram_tensor("bt", [INTER, 1], F32, kind="ExternalInput").ap()
        bp_ext = nc.dram_tensor("bp", [INTER, 1], F32, kind="ExternalInput").ap()
        bg_ext = nc.dram_tensor("bg", [1, OUT], F32, kind="ExternalInput").ap()
    out_ext = nc.dram_tensor("out", [OUT, T, V], BF16, kind="ExternalOutput").ap()

    cnt_glob = float(NCORES * T * V)

    with tile.TileContext(nc) as tc:
        with (
            tc.tile_pool(name="consts", bufs=1) as consts,
            tc.tile_pool(name="xin", bufs=2) as xin,
            tc.tile_pool(name="thp", bufs=4) as thp,
            tc.tile_pool(name="ep", bufs=8) as ep,
            tc.tile_pool(name="gp", bufs=8) as gp,
            tc.tile_pool(name="esp", bufs=6) as esp,
            tc.tile_pool(name="zp", bufs=2) as zp,
            tc.tile_pool(name="small", bufs=1) as small,
            tc.tile_pool(name="pp_th", bufs=2, space="PSUM") as pp_th,
            tc.tile_pool(name="pp_sg", bufs=3, space="PSUM") as pp_sg,
            tc.tile_pool(name="pp_z", bufs=1, space="PSUM") as pp_z,
            tc.tile_pool(name="pp_u", bufs=2, space="PSUM") as pp_u,
            tc.tile_pool(name="dram", bufs=1, space="DRAM") as dram,
        ):
            # ---- constants ----
            wt_sb = [consts.tile([P, INTER], BF16, tag=f"wt{k}", name=f"wt_sb{k}") for k in range(2)]
            wp_sb = [consts.tile([P, INTER], BF16, tag=f"wp{k}", name=f"wp_sb{k}") for k in range(2)]
            wg_sb = [consts.tile([P, OUT], BF16, tag=f"wg{k}", name=f"wg_sb{k}") for k in range(2)]
            for k in range(2):
                nc.sync.dma_start(out=wt_sb[k][:], in_=wt_ext[k])
                nc.sync.dma_start(out=wp_sb[k][:], in_=wp_ext[k])
                nc.sync.dma_start(out=wg_sb[k][:], in_=wg_ext[k])
            ones = consts.tile([P, P], BF16, tag="ones")
            nc.vector.memset(ones[:], 1.0)
            gb_sb = consts.tile([P, 4], F32, tag="gb")
            nc.sync.dma_start(out=gb_sb[:], in_=gb_ext[:])
            eps_sb = consts.tile([P, 1], F32, tag="eps")
            nc.vector.memset(eps_sb[:], BN_EPS)
            warm = consts.tile([P, 1], F32, tag="warm")
            nc.scalar.activation(warm[:], eps_sb[:], mybir.ActivationFunctionType.Exp)
            if with_bias:
                bt_sb = consts.tile([INTER, 1], F32, tag="bt")
                bp_sb = consts.tile([INTER, 1], F32, tag="bp")
                bg_sb = consts.tile([P, OUT], F32, tag="bg")
                nc.sync.dma_start(out=bt_sb[:], in_=bt_ext[:])
                nc.sync.dma_start(out=bp_sb[:], in_=bp_ext[:])
                nc.sync.dma_start(out=bg_sb[:], in_=bg_ext.to_broadcast([P, OUT]))

            # per-channel running stats (bn_stats 6-tuples per t-slice and o-chunk)
            stats_acc_t = consts.tile([P, T, 2, 6], F32, tag="stats", name="stats_acc_t")

            # all phase-1 outputs stay SBUF-resident (no DRAM scratch round-trip)
            obig = {
                (oc, tb): consts.tile([P, T_BLK, V], BF16, tag=f"obig{oc}_{tb}",
                                      name=f"obig{oc}_{tb}")
                for oc in range(2) for tb in range(N_TB)
            }

            payA = small.tile([P, 4], F32, tag="payA")
            payB = small.tile([P, 4], F32, tag="payB")
            payAB = small.tile([P, 4], F32, tag="payAB")
            payABC = small.tile([P, 4], F32, tag="payABC")
            pay_blk = {
                tag: small.tile([P, 4], F32, tag=f"payblk{tag}", name=f"payblk{tag}")
                for tag in ("A", "B", "C", "D")
            }

            # tiny warm-up AllReduce, issued early in phase 1: pre-stages the
            # collective path and absorbs cross-core launch skew while the
            # compute engines are busy, so the real stats AllReduce at the
            # end pays minimal latency
            warm_pay = dram.tile([P, 1], F32, name="warm_pay")
            warm_red = dram.tile([P, 1], F32, name="warm_red")

            def warm_collective(src):
                # src controls WHEN this fires: the collective queue idles
                # until src's producer has run, so passing phase-1 data
                # pins the sync point into the middle of phase 1
                nc.sync.dma_start(out=warm_pay[:], in_=src)
                nc.gpsimd.collective_compute(
                    "AllReduce",
                    mybir.AluOpType.add,
                    replica_groups=[list(range(NCORES))],
                    ins=[warm_pay.opt()],
                    outs=[warm_red.opt()],
                )

            stats_mv = {}

            def stats_aggr(key, t0, t1, oc):
                mv = small.tile([P, 2], F32, tag=f"mv{t0}_{oc}", name=f"mv{t0}_{oc}")
                nc.vector.bn_aggr(out=mv[:], in_=stats_acc_t[:, t0:t1, oc, :])
                stats_mv[key] = mv

            def stats_scale(pay, key, t0, t1, oc):
                # pay[:, oc] = sum, pay[:, 2+oc] = sumsq over t in [t0, t1)
                cnt = float((t1 - t0) * V)
                mv = stats_mv[key]
                t1_ = small.tile([P, 1], F32, tag=f"tmp1{t0}_{oc}", name=f"t1_{t0}_{oc}")
                t2_ = small.tile([P, 1], F32, tag=f"tmp2{t0}_{oc}", name=f"t2_{t0}_{oc}")
                nc.vector.tensor_scalar_mul(pay[:, oc : oc + 1], mv[:, 0:1], cnt)
                nc.vector.tensor_tensor(
                    t1_[:], mv[:, 0:1], mv[:, 0:1], mybir.AluOpType.mult
                )
                nc.vector.tensor_tensor(
                    t2_[:], mv[:, 1:2], t1_[:], mybir.AluOpType.add
                )
                nc.vector.tensor_scalar_mul(pay[:, 2 + oc : 3 + oc], t2_[:], cnt)

            pending = [None]

            def flush_pending():
                if pending[0] is None:
                    return
                z_ps, u_pss, ptb, ptt, pt = pending[0]
                pending[0] = None
                zinv = zp.tile([P, V], F32, tag="zinv")
                nc.vector.reciprocal_approx_fast(out=zinv[:], in_=z_ps[:])
                for oc in range(2):
                    o_ap = obig[(oc, ptb)][:, ptt, :]
                    nc.vector.tensor_tensor(
                        o_ap, u_pss[oc][:], zinv[:], mybir.AluOpType.mult
                    )
                    nc.vector.bn_stats(
                        out=stats_acc_t[:, pt, oc, :], in_=o_ap
                    )

            # ---- phase 1 ----
            for tb in range(N_TB):
                xg = [xin.tile([P, T_BLK, V], BF16, tag=f"xg{k}", name=f"xg{k}") for k in range(2)]
                if tb == 0:
                    # fine-grained first block so the first matmul starts as
                    # soon as the first t-slice lands instead of after ~1MB
                    for tt in range(T_BLK):
                        for k in range(2):
                            nc.sync.dma_start(
                                out=xg[k][:, tt, :],
                                in_=x_ext[k * P : (k + 1) * P, tt, :],
                            )
                else:
                    for k in range(2):
                        nc.sync.dma_start(
                            out=xg[k][:],
                            in_=x_ext[k * P : (k + 1) * P, tb * T_BLK : (tb + 1) * T_BLK, :],
                        )
                for tt in range(T_BLK):
                    t = tb * T_BLK + tt
                    xt = [xg[k][:, tt, :] for k in range(2)]

                    # thetaT / phiT : [INTER, V]
                    th_ps = pp_th.tile([P, V], F32, tag="th")
                    ph_ps = pp_th.tile([P, V], F32, tag="th")
                    for k in range(2):
                        nc.tensor.matmul(
                            th_ps[:], lhsT=wt_sb[k][:], rhs=xt[k],
                            start=(k == 0), stop=(k == 1),
                        )
                    for k in range(2):
                        nc.tensor.matmul(
                            ph_ps[:], lhsT=wp_sb[k][:], rhs=xt[k],
                            start=(k == 0), stop=(k == 1),
                        )
                    tp_sb = thp.tile([P, 2, V], BF16, tag="th_sb")
                    if with_bias:
                        nc.scalar.activation(
                            tp_sb[:, 0, :], th_ps[:],
                            mybir.ActivationFunctionType.Identity,
                            bias=bt_sb[:, 0:1],
                        )
                        nc.scalar.activation(
                            tp_sb[:, 1, :], ph_ps[:],
                            mybir.ActivationFunctionType.Identity,
                            bias=bp_sb[:, 0:1],
                        )
                    else:
                        # split across engines: th on ACT, ph on DVE
                        nc.scalar.copy(tp_sb[:, 0, :], th_ps[:])
                        nc.vector.tensor_copy(tp_sb[:, 1, :], ph_ps[:])
                    th_sb = tp_sb[:, 0, :]
                    ph_sb = tp_sb[:, 1, :]

                    # scores^T chunks + exp; chunk 3 stays at its true 84 rows
                    # (U matmuls use K=wsz, so no zero padding is ever read)
                    e_sb = []
                    for wc, (w0, wsz) in enumerate(WCH):
                        s_ps = pp_sg.tile([P, V], F32, tag="sg")
                        nc.tensor.matmul(
                            s_ps[:wsz], lhsT=ph_sb[:, w0 : w0 + wsz], rhs=th_sb,
                            start=True, stop=True,
                        )
                        e_t = ep.tile([P, V], BF16, tag="e")
                        nc.scalar.activation(
                            e_t[:wsz], s_ps[:wsz],
                            mybir.ActivationFunctionType.Exp, scale=SCALE,
                        )
                        e_sb.append(e_t)

                    # g chunks: [w, OUT], two chunks share one PSUM bank
                    g_sb = []
                    for pair in range(2):
                        gp_ps = pp_sg.tile([P, 2, OUT], F32, tag="sg")
                        for j in range(2):
                            w0, wsz = WCH[pair * 2 + j]
                            for k in range(2):
                                nc.tensor.matmul(
                                    gp_ps[:wsz, j, :],
                                    lhsT=xt[k][:, w0 : w0 + wsz], rhs=wg_sb[k][:],
                                    start=(k == 0), stop=(k == 1),
                                )
                        g_t = gp.tile([P, 2, OUT], BF16, tag="g")
                        if with_bias:
                            for j in range(2):
                                wsz = WCH[pair * 2 + j][1]
                                nc.vector.tensor_tensor(
                                    g_t[:wsz, j, :], gp_ps[:wsz, j, :], bg_sb[:wsz],
                                    mybir.AluOpType.add,
                                )
                        else:
                            # one full-pair ACT copy (rows past 84 in chunk 3 are
                            # junk but never read: U matmuls contract K=wsz)
                            nc.scalar.copy(g_t[:], gp_ps[:])
                        g_sb.append(g_t[:, 0, :])
                        g_sb.append(g_t[:, 1, :])

                    # softmax denominator: pre-fold E chunks pairwise on DVE,
                    # then a ones-matmul reduces over partitions + broadcasts
                    wsz3 = WCH[3][1]
                    e01 = esp.tile([P, V], BF16, tag="esum")
                    nc.vector.tensor_tensor(
                        e01[:], e_sb[0][:], e_sb[1][:], mybir.AluOpType.add
                    )
                    e23 = esp.tile([P, V], BF16, tag="esum")
                    # partition offsets must be 32-aligned; rows [64:84] are
                    # overwritten by the add below
                    nc.vector.tensor_copy(e23[64:], e_sb[2][64:])
                    nc.vector.tensor_tensor(
                        e23[:wsz3], e_sb[2][:wsz3], e_sb[3][:wsz3],
                        mybir.AluOpType.add,
                    )
                    esum = esp.tile([P, V], BF16, tag="esum")
                    nc.vector.tensor_tensor(
                        esum[:], e01[:], e23[:], mybir.AluOpType.add
                    )

                    # normalization + stats of the PREVIOUS t-slice: deferring
                    # these one iteration keeps the DVE FIFO from head-blocking
                    # on the z-matmul (its z_ps input is long done by now)
                    flush_pending()

                    # U^T = A_unnorm @ g : [OUT(2x128), V]
                    u_pss = []
                    for oc in range(2):
                        u_ps = pp_u.tile([P, V], F32, tag="u")
                        for wc, (w0, wsz) in enumerate(WCH):
                            nc.tensor.matmul(
                                u_ps[:],
                                lhsT=g_sb[wc][:wsz, oc * P : (oc + 1) * P],
                                rhs=e_sb[wc][:wsz, :],
                                start=(wc == 0), stop=(wc == 3),
                            )
                        u_pss.append(u_ps)

                    # z-matmul after the U matmuls: PE is in-order, so a wait
                    # on esum here no longer blocks the U matmuls
                    z_ps = pp_z.tile([P, V], F32, tag="z")
                    nc.tensor.matmul(
                        z_ps[:], lhsT=ones[:], rhs=esum[:], start=True, stop=True
                    )
                    pending[0] = (z_ps, u_pss, tb, tt, t)

                    if t == T_BLK - 1:
                        warm_collective(eps_sb[:])
                    # aggregate stats in 16-t blocks while phase 1 continues,
                    # staggered across t iterations to avoid DVE bursts
                    for b0, b1, tag, base in (
                        (0, 16, "A", 17),
                        (16, 32, "B", 33),
                        (32, 48, "C", 49),
                    ):
                        if t == base:
                            stats_aggr(f"{tag}0", b0, b1, 0)
                        elif t == base + 2:
                            stats_aggr(f"{tag}1", b0, b1, 1)
                        elif t == base + 4:
                            stats_scale(pay_blk[tag], f"{tag}0", b0, b1, 0)
                        elif t == base + 6:
                            stats_scale(pay_blk[tag], f"{tag}1", b0, b1, 1)
                    if t == 58:
                        nc.vector.tensor_tensor(
                            payAB[:], pay_blk["A"][:], pay_blk["B"][:],
                            mybir.AluOpType.add,
                        )
                    elif t == 60:
                        nc.vector.tensor_tensor(
                            payA[:], payAB[:], pay_blk["C"][:],
                            mybir.AluOpType.add,
                        )

            flush_pending()

            # ---- BN stats tail: aggregate t in [48,64), sum, all-reduce ----
            for oc in range(2):
                stats_aggr(f"E{oc}", 48, T, oc)
                stats_scale(payB, f"E{oc}", 48, T, oc)
            pay = small.tile([P, 4], F32, tag="pay")
            nc.vector.tensor_tensor(pay[:], payA[:], payB[:], mybir.AluOpType.add)

            pay_dram = dram.tile([P, 4], F32)
            red_dram = dram.tile([P, 4], F32)
            nc.sync.dma_start(out=pay_dram[:], in_=pay[:])
            nc.gpsimd.collective_compute(
                "AllReduce",
                mybir.AluOpType.add,
                replica_groups=[list(range(NCORES))],
                ins=[pay_dram.opt()],
                outs=[red_dram.opt()],
            )
            red = small.tile([P, 4], F32, tag="red")
            nc.sync.dma_start(out=red[:], in_=red_dram[:])

            # mean = sum/N ; var = sumsq/N - mean^2 ; s = gamma/sqrt(var+eps)
            # t = beta - mean*s
            mean_g = small.tile([P, 2], F32, tag="mean_g")
            ex2 = small.tile([P, 2], F32, tag="ex2")
            var_g = small.tile([P, 2], F32, tag="var_g")
            rstd = small.tile([P, 2], F32, tag="rstd")
            s_vec = small.tile([P, 2], F32, tag="s_vec")
            t_vec = small.tile([P, 2], F32, tag="t_vec")
            nc.vector.tensor_scalar_mul(mean_g[:], red[:, 0:2], 1.0 / cnt_glob)
            nc.vector.tensor_scalar_mul(ex2[:], red[:, 2:4], 1.0 / cnt_glob)
            nc.vector.tensor_tensor(
                var_g[:], mean_g[:], mean_g[:], mybir.AluOpType.mult
            )
            nc.vector.tensor_tensor(
                var_g[:], ex2[:], var_g[:], mybir.AluOpType.subtract
            )
            nc.scalar.activation(
                rstd[:], var_g[:], mybir.ActivationFunctionType.Sqrt,
                bias=eps_sb[:, 0:1],
            )
            nc.vector.reciprocal(out=rstd[:], in_=rstd[:])
            nc.vector.tensor_tensor(s_vec[:], rstd[:], gb_sb[:, 0:2], mybir.AluOpType.mult)
            nc.vector.tensor_tensor(t_vec[:], mean_g[:], s_vec[:], mybir.AluOpType.mult)
            nc.vector.tensor_tensor(
                t_vec[:], gb_sb[:, 2:4], t_vec[:], mybir.AluOpType.subtract
            )

            # ---- phase 2: SBUF-resident o -> affine in place -> bf16 out ----
            # the affine runs in place on obig, so the output DMA streams
            # straight from it with no staging tiles in between
            half = T_BLK // 2
            for tb in range(N_TB):
                for oc in range(2):
                    t0 = tb * T_BLK
                    tin = obig[(oc, tb)]
                    for h in range(2):
                        nc.vector.tensor_scalar(
                            tin[:, h * half : (h + 1) * half, :],
                            tin[:, h * half : (h + 1) * half, :],
                            s_vec[:, oc : oc + 1], t_vec[:, oc : oc + 1],
                            mybir.AluOpType.mult, mybir.AluOpType.add,
                        )
                        nc.sync.dma_start(
                            out=out_ext[oc * P : (oc + 1) * P,
                                        t0 + h * half : t0 + (h + 1) * half, :],
                            in_=tin[:, h * half : (h + 1) * half, :],
                        )

    nc.compile()
    return nc


def _get_nc(with_bias: bool):
    key = with_bias
    if key not in _CACHE:
        _CACHE[key] = _build(with_bias)
    return _CACHE[key]


def _ensure_ntff_hook():
    import sys, types
    import antenv

    if "antenv.axon_hooks" not in sys.modules:
        mod = types.ModuleType("antenv.axon_hooks")
        _h = [None]
        mod.set_axon_ntff_profile_hook = lambda h: _h.__setitem__(0, h)
        mod.get_axon_ntff_profile_hook = lambda: _h[0]
        sys.modules["antenv.axon_hooks"] = mod
        antenv.axon_hooks = mod
    mod = sys.modules["antenv.axon_hooks"]
    if mod.get_axon_ntff_profile_hook() is None:
        try:
            from trn_agent_boot.trn_boot import _ntff_profile_via_ctypes

            mod.set_axon_ntff_profile_hook(
                _ntff_profile_via_ctypes("/opt/axon/libaxon_pjrt.so")
            )
        except Exception:
            pass


def kernel(x, W_theta, b_theta, W_phi, b_phi, W_g, b_g, bn_gamma, bn_beta):
    global LAST_EXEC_NS
    x = np.asarray(x, dtype=np.float32)
    with_bias = bool(
        np.any(np.asarray(b_theta)) or np.any(np.asarray(b_phi)) or np.any(np.asarray(b_g))
    )

    x_bf = x.astype(ml_dtypes.bfloat16)  # (B, C, T, V)
    wt = np.ascontiguousarray(
        np.asarray(W_theta, dtype=np.float32).T.astype(ml_dtypes.bfloat16).reshape(2, P, INTER)
    )
    wp = np.ascontiguousarray(
        np.asarray(W_phi, dtype=np.float32).T.astype(ml_dtypes.bfloat16).reshape(2, P, INTER)
    )
    wg = np.ascontiguousarray(
        np.asarray(W_g, dtype=np.float32).T.astype(ml_dtypes.bfloat16).reshape(2, P, OUT)
    )
    gamma = np.asarray(bn_gamma, dtype=np.float32).reshape(2, P).T  # [128, 2]
    beta = np.asarray(bn_beta, dtype=np.float32).reshape(2, P).T
    gb = np.ascontiguousarray(np.concatenate([gamma, beta], axis=1))  # [128, 4]

    nc = _get_nc(with_bias)

    in_maps = []
    for b in range(NCORES):
        m = {
            "x": np.ascontiguousarray(x_bf[b]),
            "wt": wt,
            "wp": wp,
            "wg": wg,
            "gb": gb,
        }
        if with_bias:
            m["bt"] = np.asarray(b_theta, dtype=np.float32).reshape(INTER, 1)
            m["bp"] = np.asarray(b_phi, dtype=np.float32).reshape(INTER, 1)
            m["bg"] = np.asarray(b_g, dtype=np.float32).reshape(1, OUT)
        in_maps.append(m)

    if TRACE:
        _ensure_ntff_hook()
    r = run_bass_kernel_spmd(nc, in_maps, list(range(NCORES)), trace=TRACE)
    LAST_EXEC_NS = r.exec_time_ns

    out = np.stack([r.results[b]["out"] for b in range(NCORES)], axis=0)
    return out.astype(np.float32)
